# revision 32
# baseline (speedup 1.0000x reference)
"""Averaged Hausdorff loss kernel for 8 Trainium2 NeuronCores.

Math: for row-sharded blocks, d2[i,j] = |s1_i|^2 + |s2_j|^2 - 2<s1_i, s2_j>
is computed as a single K=13 matmul with augmented operands.  Inputs are
split hi/lo into two fp16 parts (x = xh + xl exact to ~2^-22 rel), so the
matmul runs at full PE rate (1 cycle/row vs 4 for fp32) while keeping
~fp32 accuracy: the K rows pair up as
    xh*(-2yh) (3) + xh*(-2yl) (3) + xl*(-2yh) (3) + nh*1 + nl*1 + 1*n'h + 1*n'l
so each PSUM tile holds squared distances directly.  min_j sqrt(d2) =
sqrt(min_j d2), so only the [128,1] row-mins ever leave the device; the
sqrt + mean (or max) run on host in fp64.

Sharding: core r owns rows [r*1024, (r+1)*1024) of set1 (reduced against
all of set2) and the same rows of set2 (reduced against all of set1).
Both directions are free-dim row-min reductions; no collectives needed.

Consumption engines: the DVE's two read ports (1 PSUM + 1 SBUF, 32b each)
are the kernel bottleneck.  Two custom fused DVE ops drain PSUM:
  MINSCAN_ANT  - running-min scan over min(in0_psum_f32, in1_sbuf_f32):
                 2 values/cycle, the SBUF side pre-copied by ScalarE.
  MINSCAN3_ANT - same, but in1 is a *packed fp16 pair* read through the
                 SRC_1 + SRC_1_HI crossbar lanes with a stride-2 AP, so
                 one 32b SBUF read carries two values: 3 values/cycle.
                 ScalarE converts fp32 PSUM -> packed fp16 (1 elem/cyc,
                 read-bound, same cost as the fp32 copy).
MINSCAN3 is disabled (MIXED_B below): 16-bit reads through the DVE PSUM
port fault TRN2 silicon.  All rows run the MINSCAN quad schedule; the
first block splits its first quad in two so the DVE starts ~1us sooner.
"""

import sys

sys.path.insert(0, "/opt/trn_rl_repo")

import numpy as np

N_CORES = 8
N = 8192          # set1 rows
M = 8192          # set2 rows
D = 3
ROWS_PER_CORE = N // N_CORES          # 1024
BLOCKS = ROWS_PER_CORE // 128         # 8 row-blocks of 128
CHUNK = 512                           # matmul free dim (one PSUM bank)
N_CHUNKS = M // CHUNK                 # 16 chunks per block row
K = 13                                # augmented contraction dim
FP32_MAX = 3.4e38
# Block indices using the MINSCAN3 schedule.  Kept empty: the 2X_1PORT
# program (even shaped exactly like the stock TENSOR_TENSOR 2X slot, with
# perf_max=1 plumbed through) reliably faults the engine with
# NRT_EXEC_UNIT_UNRECOVERABLE — evidence that 16-bit reads through the
# DVE's PSUM port are a TRN3-only feature (TRN2 matmul cannot produce
# 16-bit PSUM, and bass gates that path to TRN3 for exactly this reason).
# Every row therefore uses the proven MINSCAN quad schedule; the MINSCAN3
# machinery below is inert but kept for a future TRN3 port.
MIXED_B = ()
GROUPS_MIXED = (4, 3, 3, 3, 3)
GROUPS_PURE = (4, 4, 4, 4)
MAX_GROUPS = 5

_compiled = None


def _register_min2():
    """Fused DVE op (per-NEFF uop table): a running min-scan over
    min(in0[p,k], in1[p,k]) seeded from s0, written through a stride-0
    broadcast AP so the destination cell holds the total min.  Consumes two
    fp32 streams (PSUM + SBUF ports) at 2 elements/cycle."""
    from concourse import dve_ops
    from concourse.dve_spec import Spec, Src0, Src1, minn, C0, AluOp, lower, scan
    from concourse.dve_spec import _has_src1 as has_src1
    from concourse.dve_uop import DveOpSpec

    def _ref(in0, in1, c0, c1, c2):
        b = np.minimum(in0.astype(np.float32), in1.astype(np.float32))
        P = b.shape[0]
        init = (
            np.asarray(c0, np.float32).reshape(-1, 1)
            if np.ndim(c0)
            else np.full((P, 1), c0, np.float32)
        )
        flat = np.minimum.accumulate(
            np.concatenate([init, b.reshape(P, -1)], axis=1), axis=-1
        )[:, 1:]
        return flat.reshape(b.shape)

    name = "MINSCAN_ANT"
    spec = Spec(body=scan(AluOp.MIN, minn(Src0, Src1), init=C0), reference=_ref)
    if name in dve_ops._SUB_OPCODE_FOR_NAME:
        return next(op for op in dve_ops.OPS if op.name == name)
    op = dve_ops.DveOp(name, spec, subdim=False, uops_sha={})
    dve_ops.OPS.append(op)
    dve_ops._SUB_OPCODE_FOR_NAME[name] = (
        dve_ops._CUSTOM_DVE_ROW_BASE + len(dve_ops.OPS) - 1
    )
    assert dve_ops._SUB_OPCODE_FOR_NAME[name] < 0x20
    dve_ops.CUSTOM_DVE_SPECS[name] = spec
    for ver in ("v3", "v4"):
        compiled = DveOpSpec(
            name=name,
            opcode=dve_ops.get_dve_sub_opcode(name),
            uops=lower(spec, ver=ver),
            rd1_en=has_src1(spec),
        )
        op.uops_sha[ver] = compiled.sha(ver)
    return op


def _min3_uops(port0_hi):
    """Hand-built uop programs for MINSCAN3_ANT (v3/TRN2).

    Runs in 2X_1PORT mode: all operands are 16-bit step-1, so each 32b
    port read carries two elements.  in0 is a bf16 *bitcast* of the fp32
    PSUM tile — element 2k is the fp32's low mantissa half (garbage,
    ignored), element 2k+1 (SRC_0_HI) is its high half, i.e. the bf16
    truncation of d2 (≤2^-8 one-sided rel err, harmless under sqrt and
    the 2e-2 budget).  in1 is packed fp16 pairs (SRC_1 + SRC_1_HI).  Per
    cycle: running = MIN(running, MIN(MIN(port0_val, SRC_1), SRC_1_HI))
    -> one fp32 PSUM value + two converted values = 3 values/cycle.
    Mirrors the lower()-generated MINSCAN structure: a 1-cycle seed uop
    loads C0 into the scan block's self-referencing flop, then the steady
    uop streams until SRC_TENSOR_DONE, writing the running value through
    the broadcast out AP every cycle."""
    from concourse.dve_spec import AluOp
    from concourse.dve_uop import (
        UopConfig,
        InpSel,
        AluInp,
        DelayInp,
        Trigger,
        OutSel,
        OutPath,
        ENABLE,
    )

    def mk(seed):
        # Mirrors the stock TENSOR_TENSOR 2X_1PORT program's shape (lane 0
        # active, input_enable=0xf, requires on, packed lo+hi writes with
        # the hi half sourced from DELAY_0) with two changes: the port-0
        # lane muxes SRC_0_HI instead of SRC_0, and a seed state preloads
        # C0 into the scan block's self-referencing flop.
        u = UopConfig()
        u.enable_input(
            InpSel.SRC_0_HI if port0_hi else InpSel.SRC_0, 0
        )                                    # blk0 alu input A
        u.enable_input(InpSel.SRC_1, 1)      # blk0 PREV_DELAY_0
        u.enable_input(InpSel.SRC_1_HI, 2)   # blk0 PREV_DELAY_1
        u.enable_input(InpSel.CONST_0, 3)    # blk0 PREV_DELAY_2
        b = u.datapath_config
        # blk0: min(port0, src1_lo); carry src1_hi and C0 forward
        b[0].enable_alu(AluOp.MIN, AluInp.PREV_ALU_OUT, AluInp.PREV_DELAY_0)
        b[0].pass_through_delay(1, 2)
        # blk1: min(blk0, src1_hi); carry C0
        b[1].enable_alu(AluOp.MIN, AluInp.PREV_ALU_OUT, AluInp.PREV_DELAY_1)
        b[1].pass_through_delay(2)
        if seed:
            # blk2 out-flop <- C0: seeds the steady state's CURR_ALU_OUT.
            # blk1's element-0 result is consumed by the steady state's
            # blk2 one cycle later via PREV_ALU_OUT, so nothing is lost.
            b[2].enable_alu(AluOp.BYPASS, AluInp.PREV_DELAY_2, AluInp.PREV_DELAY_2)
        else:
            # blk2: running = min(running, blk1)
            b[2].enable_alu(AluOp.MIN, AluInp.CURR_ALU_OUT, AluInp.PREV_ALU_OUT)
        # blk3 loads the scan value into delay0 (stock writes the hi half
        # from DELAY_0); blk3..7 also carry it down the ALU bypass chain.
        b[3].enable_alu(AluOp.BYPASS, AluInp.PREV_ALU_OUT, AluInp.PREV_ALU_OUT)
        b[3].enable_delay_from_src(DelayInp.PREV_ALU_OUT, 0)
        for i in range(4, 8):
            b[i].enable_alu(AluOp.BYPASS, AluInp.PREV_ALU_OUT, AluInp.PREV_ALU_OUT)
            b[i].pass_through_delay(0)
        u.enable_output(OutSel.ALU_OUT, OutPath.WR0_LO)
        u.enable_output(OutSel.DELAY_0, OutPath.WR0_HI)
        u.require_inp0 = ENABLE
        u.require_inp1 = ENABLE
        if seed:
            u.repeat_count = 1
            u.trigger = (Trigger.COUNT, Trigger.NONE, Trigger.NONE)
            u.next_uop = (1, 0, 0)
        else:
            u.trigger = (Trigger.SRC_TENSOR_DONE, Trigger.NONE, Trigger.NONE)
            u.next_uop = (0, 0, 0)
        return u

    return [mk(seed=True), mk(seed=False)]


def _register_min3():
    """Register MINSCAN3_ANT: like MINSCAN_ANT but in1 carries packed fp16
    pairs (2 values per 32b read) -> 3 values consumed per DVE cycle."""
    from concourse import dve_ops
    from concourse.dve_spec import Spec, Src0, Src1, minn, C0, AluOp, scan
    from concourse.dve_uop import DveOpSpec

    def _ref(in0, in1, c0, c1, c2):
        # Simulation-only approximation: the hi fp16 halves read by the
        # hardware are not visible through the stride-2 AP, so this
        # reference covers only the lo stream.  Never compared on the
        # axon/PJRT execution path used here.
        b = np.minimum(in0.astype(np.float32), in1.astype(np.float32))
        P = b.shape[0]
        init = np.full((P, 1), c0, np.float32)
        flat = np.minimum.accumulate(
            np.concatenate([init, b.reshape(P, -1)], axis=1), axis=-1
        )[:, 1:]
        return flat.reshape(b.shape)

    name = "MINSCAN3_ANT"
    if name in dve_ops._SUB_OPCODE_FOR_NAME:
        return next(op for op in dve_ops.OPS if op.name == name)
    spec = Spec(body=scan(AluOp.MIN, minn(Src0, Src1), init=C0), reference=_ref)
    op = dve_ops.DveOp(name, spec, subdim=False, uops_sha={})
    dve_ops.OPS.append(op)
    dve_ops._SUB_OPCODE_FOR_NAME[name] = (
        dve_ops._CUSTOM_DVE_ROW_BASE + len(dve_ops.OPS) - 1
    )
    assert dve_ops._SUB_OPCODE_FOR_NAME[name] < 0x20
    dve_ops.CUSTOM_DVE_SPECS[name] = spec
    spec3 = DveOpSpec(
        name=name,
        opcode=dve_ops.get_dve_sub_opcode(name),
        # REGULAR slot is unreachable (operand dtypes/steps deterministically
        # select 2X_1PORT, and the slot exists) but must be populated.
        uops=_min3_uops(port0_hi=False),
        uops_2x=_min3_uops(port0_hi=True),
        perf_max=1,
        rd1_en=True,
    )
    spec3.validate("v3")
    # Pre-seed the compile cache so DveOp.compile() returns the hand-built
    # program (lower() cannot express the SRC_1_HI input lane).
    dve_ops._COMPILE_CACHE[(name, "v3")] = spec3
    op.uops_sha["v3"] = spec3.sha("v3")
    return op


def _build_program():
    import concourse.tile as tile
    from concourse import bacc, bass_isa, mybir

    min2 = _register_min2()
    min3 = _register_min3()

    # _custom_dve does not expose the instruction's perf_max field (byte-36
    # [7:6], the highest engine-reachable perf-mode slot).  Without it the
    # engine clamps to the REGULAR uop slot and never runs the 2X_1PORT
    # program MINSCAN3 relies on.  Inject it at construction.
    _orig_ctor = bass_isa.InstCustomDveAnt

    def _ctor(*a, **kw):
        if kw.get("op_name") == "MINSCAN3_ANT":
            kw.setdefault("perf_max", 1)
        return _orig_ctor(*a, **kw)

    bass_isa.InstCustomDveAnt = _ctor

    nc = bacc.Bacc("TRN2", target_bir_lowering=False, debug=False)
    f32 = mybir.dt.float32
    f16 = mybir.dt.float16

    KR = 32 + K   # SBUF operand stack height (replicas at rows 0..12, 32..44)
    lhs1_d = nc.dram_tensor("lhs1", [2 * K, ROWS_PER_CORE], f16, kind="ExternalInput")
    rhs2_d = nc.dram_tensor("rhs2", [2 * K, M], f16, kind="ExternalInput")
    lhs2_d = nc.dram_tensor("lhs2", [2 * K, ROWS_PER_CORE], f16, kind="ExternalInput")
    rhs1_d = nc.dram_tensor("rhs1", [2 * K, N], f16, kind="ExternalInput")
    out_d = nc.dram_tensor("out", [128, 2 * BLOCKS], f32, kind="ExternalOutput")

    with tile.TileContext(nc) as tc:
        with (
            tc.tile_pool(name="ops", bufs=1) as ops,
            tc.tile_pool(name="ps_keep", bufs=2, space="PSUM") as ps_keep,
            tc.tile_pool(name="ps_copy", bufs=2, space="PSUM") as ps_copy,
            tc.tile_pool(name="scopy", bufs=7) as scopy,
            tc.tile_pool(name="small", bufs=1) as small,
        ):
            # Operand stacks come pre-replicated from the host at base
            # partitions 0 and 32, so consecutive matmuls can target
            # different PE row-groups: LDWEIGHTS for one row-group overlaps
            # the matmul streaming in the other, and alternating-group
            # matmul pairs stream concurrently.  The first 2048 columns
            # live in separate "early" tiles so the first matmuls start as
            # soon as the small early DMAs land.
            E = 4 * CHUNK
            lhs1 = ops.tile([KR, ROWS_PER_CORE], f16, tag="lhs1")
            lhs2 = ops.tile([KR, ROWS_PER_CORE], f16, tag="lhs2")
            rhs2e = ops.tile([KR, E], f16, tag="rhs2e")
            rhs2 = ops.tile([KR, M - E], f16, tag="rhs2")
            rhs1e = ops.tile([KR, E], f16, tag="rhs1e")
            rhs1 = ops.tile([KR, N - E], f16, tag="rhs1")

            # Each replica only serves chunks of its parity (even chunks
            # read row-group 0, odd chunks row-group 32), so the bulk loads
            # bring only that half of the columns into each replica —
            # halves DMA traffic.
            def half(ap2d, parity):
                r3 = ap2d.rearrange("k (n s) -> k n s", s=2 * CHUNK)
                return (
                    r3[:, :, 0:CHUNK]
                    if parity == 0
                    else r3[:, :, CHUNK : 2 * CHUNK]
                )

            # Critical path: the first matmuls need lhs1 + rhs2e.  All four
            # early loads ride the sync (SP) HWDGE ring — ~0.6us first-byte,
            # and the ACT ring is blocked early by the ~1.3us ACT_TABLE_LOAD
            # walrus schedules before the first scalar COPY.  The gpsimd
            # SWDGE path needs ~6us of Q7 spin-up, so only tensors needed
            # later go there.
            nc.sync.dma_start(lhs1[0:K, :], lhs1_d[0:K, :])
            nc.sync.dma_start(rhs2e[0:K, :], rhs2_d[0:K, 0:E])
            nc.sync.dma_start(lhs1[32 : 32 + K, :], lhs1_d[K : 2 * K, :])
            nc.sync.dma_start(rhs2e[32 : 32 + K, :], rhs2_d[K : 2 * K, 0:E])
            for r, g in ((0, 0), (1, 32)):
                rs = slice(r * K, (r + 1) * K)
                nc.gpsimd.dma_start(lhs2[g : g + K, :], lhs2_d[rs, :])
                nc.gpsimd.dma_start(
                    half(rhs1e[g : g + K, :], r), half(rhs1_d[rs, 0:E], r)
                )

            def emit_rest_dmas():
                for r, g in ((0, 0), (1, 32)):
                    rs = slice(r * K, (r + 1) * K)
                    nc.sync.dma_start(
                        half(rhs2[g : g + K, :], r), half(rhs2_d[rs, E:M], r)
                    )
                    nc.gpsimd.dma_start(
                        half(rhs1[g : g + K, :], r), half(rhs1_d[rs, E:M], r)
                    )

            rowmin = small.tile([128, 2 * BLOCKS], f32, tag="rowmin")
            rowpart = small.tile([128, 2 * BLOCKS, 5], f32, tag="rowpart")
            # Not every cell is written (only the split first block uses all
            # 5 group cells); seed them all above any d2.
            nc.gpsimd.memset(rowpart[:], FP32_MAX)
            if MIXED_B:
                rowmin3 = small.tile([128, 2 * BLOCKS], f32, tag="rowmin3")
                # min3 triples write through a 16-bit out path; their cells
                # are fp16 (2^-11 rel rounding on d2, immaterial), padded to
                # 32b pairs because the 2X write path stores packed halves.
                rowpart3 = small.tile(
                    [128, 2 * BLOCKS, 4, 2], f16, tag="rowpart3"
                )
                nc.gpsimd.memset(rowpart3[:], 60000.0)

            for o, (lhs_s, rhs_e, rhs_r) in enumerate(
                ((lhs1, rhs2e, rhs2), (lhs2, rhs1e, rhs1))
            ):
                for b in range(BLOCKS):
                    ob = o * BLOCKS + b
                    bc = slice(b * 128, (b + 1) * 128)
                    groups = GROUPS_MIXED if b in MIXED_B else GROUPS_PURE
                    # (A (2,2,4,4,4) split of the first block was measured
                    # net-negative: the DVE starts ~0.4us earlier but then
                    # stalls ~1.3us longer — the startup ramp is bound by the
                    # cold PE's matmul rate, not by the first group's size.)

                    def mm(dst, c):
                        g = 32 * (c % 2)
                        if c * CHUNK < E:
                            src, base = rhs_e, c * CHUNK
                        else:
                            src, base = rhs_r, c * CHUNK - E
                        nc.tensor.matmul(
                            dst,
                            lhs_s[g : g + K, bc],
                            src[g : g + K, base : base + CHUNK],
                        )

                    c = 0
                    for gi, gsz in enumerate(groups):
                        # defer the bulk loads until the first group is
                        # emitted so its matmuls don't wait on them
                        if o == 0 and b == 0 and gi == 1:
                            emit_rest_dmas()
                        pk = ps_keep.tile([128, 2, CHUNK], f32, name="pk", tag="pk")
                        pc = ps_copy.tile([128, 2, CHUNK], f32, name="pc", tag="pc")
                        # pc fills before pk: the ScalarE copy (1114ns) is
                        # rate-matched with the MINSCAN (1162ns) within
                        # ~50ns, so starting it two matmuls earlier hides
                        # its completion-semaphore latency from the DVE
                        # (~150ns stalls otherwise, seen in the dir-2 phase).
                        if gsz == 2:
                            mm(pk[:, 0, :], c)
                            mm(pc[:, 0, :], c + 1)
                            sc = scopy.tile(
                                [128, 1, CHUNK], f32, name="sc", tag="sc"
                            )
                            nc.scalar.copy(sc[:], pc[:, 0:1, :])
                            cell = rowpart[:, ob, gi : gi + 1]
                            nc.vector._custom_dve(
                                min2,
                                out=cell.broadcast_to((128, 1, CHUNK)),
                                in0=pk[:, 0:1, :],
                                in1=sc[:],
                                s0=FP32_MAX,
                            )
                        elif gsz == 4:
                            for t, dst in ((0, pc), (1, pc), (2, pk), (3, pk)):
                                mm(dst[:, t % 2, :], c + t)
                            sc = scopy.tile(
                                [128, 2, CHUNK], f32, name="sc", tag="sc"
                            )
                            nc.scalar.copy(sc[:], pc[:])
                            cell = rowpart[:, ob, gi : gi + 1]
                            nc.vector._custom_dve(
                                min2,
                                out=cell.broadcast_to((128, 2, CHUNK)),
                                in0=pk[:],
                                in1=sc[:],
                                s0=FP32_MAX,
                            )
                        else:
                            # min3 triple: 1 direct chunk + 2 converted to
                            # packed fp16 (pk's second bank rides unused)
                            mm(pk[:, 0, :], c)
                            mm(pc[:, 0, :], c + 1)
                            mm(pc[:, 1, :], c + 2)
                            sc16 = scopy.tile(
                                [128, 2, CHUNK], f16, name="sc16", tag="sc16"
                            )
                            nc.scalar.copy(sc16[:], pc[:])
                            cell = rowpart3[:, ob, gi - 1, 0:1]
                            nc.vector._custom_dve(
                                min3,
                                out=cell.broadcast_to((128, 4 * CHUNK)),
                                in0=pk[:, 0, :].bitcast(mybir.dt.bfloat16),
                                in1=sc16.rearrange("p s n -> p (s n)"),
                                s0=FP32_MAX,
                            )
                        c += gsz

            nc.vector.tensor_reduce(
                rowmin[:],
                rowpart[:],
                axis=mybir.AxisListType.X,
                op=mybir.AluOpType.min,
            )
            if MIXED_B:
                nc.vector.tensor_reduce(
                    rowmin3[:],
                    rowpart3.rearrange("p o t h -> p o (t h)"),
                    axis=mybir.AxisListType.X,
                    op=mybir.AluOpType.min,
                )
                rowminf = small.tile([128, 2 * BLOCKS], f32, tag="rowminf")
                nc.vector.tensor_tensor(
                    rowminf[:], rowmin[:], rowmin3[:], op=mybir.AluOpType.min
                )
                nc.sync.dma_start(out_d[:], rowminf[:])
            else:
                nc.sync.dma_start(out_d[:], rowmin[:])

    bass_isa.InstCustomDveAnt = _orig_ctor
    nc.compile()
    return nc


def _get_program():
    global _compiled
    if _compiled is None:
        _compiled = _build_program()
    return _compiled


def _split16(v):
    """fp64 vector -> (hi, lo) fp16 with v ~= hi + lo to ~2^-22 rel."""
    hi = v.astype(np.float16)
    lo = (v - hi.astype(np.float64)).astype(np.float16)
    return hi.astype(np.float64), lo.astype(np.float64)


def _replicate(stack):
    """[13, n] -> [26, n]: two packed copies, one per PE row-group."""
    return np.ascontiguousarray(np.concatenate([stack, stack], axis=0))


def _aug_operands(s):
    """Build [26, n] lhsT and rhs operand stacks in fp16 (hi/lo split,
    replicated for the two PE row-groups)."""
    s64 = s.astype(np.float64)
    n = (s64 * s64).sum(axis=1)
    ones = np.ones(s.shape[0], dtype=np.float64)
    xh = [None] * D
    xl = [None] * D
    for d in range(D):
        xh[d], xl[d] = _split16(s64[:, d])
    nh, nl = _split16(n)
    lhs = np.stack(
        [xh[0], xh[1], xh[2], xh[0], xh[1], xh[2], xl[0], xl[1], xl[2],
         nh, nl, ones, ones]
    ).astype(np.float16)
    rhs = np.stack(
        [-2 * xh[0], -2 * xh[1], -2 * xh[2], -2 * xl[0], -2 * xl[1], -2 * xl[2],
         -2 * xh[0], -2 * xh[1], -2 * xh[2], ones, ones, nh, nl]
    ).astype(np.float16)
    return _replicate(lhs), _replicate(rhs)


def _run_device(s1, s2, trace=False):
    from concourse.bass_utils import run_bass_kernel_spmd

    nc = _get_program()
    lhs1_full, rhs1_full = _aug_operands(s1)
    lhs2_full, rhs2_full = _aug_operands(s2)

    in_maps = []
    for r in range(N_CORES):
        sl = slice(r * ROWS_PER_CORE, (r + 1) * ROWS_PER_CORE)
        in_maps.append(
            {
                "lhs1": np.ascontiguousarray(lhs1_full[:, sl]),
                "rhs2": rhs2_full,
                "lhs2": np.ascontiguousarray(lhs2_full[:, sl]),
                "rhs1": rhs1_full,
            }
        )

    # Transient NRT_EXEC_UNIT_UNRECOVERABLE failures have been observed on
    # the first execution after unrelated device activity; retry a couple
    # of times before giving up.
    last_err = None
    for _attempt in range(3):
        try:
            res = run_bass_kernel_spmd(nc, in_maps, list(range(N_CORES)), trace=trace)
            break
        except Exception as e:
            last_err = e
    else:
        raise last_err

    d1min = np.concatenate(
        [res.results[r]["out"][:, 0:BLOCKS].T.reshape(-1) for r in range(N_CORES)]
    )
    d2min = np.concatenate(
        [res.results[r]["out"][:, BLOCKS : 2 * BLOCKS].T.reshape(-1) for r in range(N_CORES)]
    )
    return d1min, d2min, res


def kernel(set1, set2, hausdorff=0, w_set1_set2=1, w_set2_set1=1, n_outputs=1):
    s1 = np.ascontiguousarray(np.asarray(set1, dtype=np.float32))
    s2 = np.ascontiguousarray(np.asarray(set2, dtype=np.float32))
    assert s1.shape == (N, D) and s2.shape == (M, D), (s1.shape, s2.shape)
    hausdorff = int(np.asarray(hausdorff))
    w12 = int(np.asarray(w_set1_set2))
    w21 = int(np.asarray(w_set2_set1))
    n_outputs = int(np.asarray(n_outputs))

    d1min, d2min, _ = _run_device(s1, s2)

    d1 = np.sqrt(np.maximum(d1min, 0.0).astype(np.float64))
    d2 = np.sqrt(np.maximum(d2min, 0.0).astype(np.float64))
    reduce = np.mean if hausdorff == 0 else np.max
    t12 = np.float32(reduce(d1)) if w12 != 0 else np.float32(0.0)
    t21 = np.float32(reduce(d2)) if w21 != 0 else np.float32(0.0)

    if n_outputs == 1:
        return np.float32(t12 + t21)
    return (t12, t21)
